# revision 1
# baseline (speedup 1.0000x reference)
"""Trainium2 Bass kernel for the attention module:

    xp      = x @ W.T + b                      # [B, E]
    scores  = einsum('be,tbe->bt', xp, enc)    # [B, T]
    attn    = softmax(scores, axis=1)
    context = einsum('bt,tbe->be', attn, enc)  # [B, E]
    out     = concat([xp, context], axis=1)    # [B, 2E]

Shapes: T=2048, B=128, D_dec=512, E=512 (fp32).

Strategy (data-parallel over batch, 8 NeuronCores, no collectives):
  - Each core owns NB=16 batches: its encoder_states shard is
    [T, 16, E] = 64 MiB, streamed from HBM exactly once in NT=16
    t-tiles of [128, 16, 512] (4 MB) -> memory-roofline bound.
  - Per tile k (flash-style, deferred softmax combine):
      prod     = enc * xp            (VectorE tensor_tensor, 4 groups)
      S[t,b]   = sum_e prod          (ScalarE activation-Copy accum_out)
      sT       = S^T                 (TensorE transpose)
      -m_k     = -rowmax(sT)         (VectorE reduce, negate)
      pT, l_k  = exp(sT - m_k) + rowsum (ScalarE activation + accum)
      p        = pT^T                (TensorE transpose)
      c_k      = per-batch sum_t p * enc via 16 masked matmuls
                 accumulating one [16, 512] PSUM tile (TensorE)
  - Final: exact softmax combine over the 16 tiles' (m_k, l_k, c_k).

This toolchain's walrus accepts AT MOST ONE semaphore wait per TPB
compute instruction, and Tile pool slot reuse emits extra release
waits.  Hence: hot buffers are allocated once and alternated manually,
and cheap "observer" ops make each engine see a new producer before
the real consumer runs, keeping every instruction at <= 1 wait.
"""

import os
import sys

import numpy as np

if "/opt/trn_rl_repo" not in sys.path and not any(
    os.path.isdir(os.path.join(p, "concourse")) for p in sys.path if p
):
    sys.path.insert(0, "/opt/trn_rl_repo")

import concourse.bass as bass
import concourse.mybir as mybir
import concourse.tile as tile
from concourse.bass_utils import run_bass_kernel_spmd
from concourse.masks import make_identity
from concourse.tile_rust import add_dep_helper

T, B, D, E = 2048, 128, 512, 512
NCORES = 8
NB = B // NCORES  # 16 local batches per core
PT = 128          # t-tile partition size
NT = T // PT      # 16 t-tiles
NC_D = D // 128   # 4 chunks of the contraction dim for the xp matmul
GRP = 2           # batches per tensor_tensor multiply group

F32 = mybir.dt.float32
AF = mybir.ActivationFunctionType
ALU = mybir.AluOpType
AX = mybir.AxisListType


def _install_drain_split():
    """This walrus rejects instructions carrying more than one semaphore
    wait.  Tile's kernel-tail drain waits on every proc's final tick in a
    single instruction; split it into one drain per wait."""
    from concourse.vector_clock import ScopedClock

    if getattr(tile.TileContext, "_drain_split_installed", False):
        return

    def _split_dab(self, tick_clock, wait_clock):
        drain_inst = self.nc.sync.drain()
        wait_clock.add_sem_waits(
            drain_inst.ins, ScopedClock({None: tick_clock.global_clock})
        )
        si = drain_inst.ins.sync_info
        if si is not None and len(si.on_wait) > 1:
            waits = list(si.on_wait)
            upds = list(si.on_update)
            drain_inst.ins.sync_info = mybir.SyncInfo(
                on_wait=[waits[0]], on_update=upds
            )
            for w in waits[1:]:
                d2 = self.nc.sync.drain()
                d2.ins.sync_info = mybir.SyncInfo(on_wait=[w], on_update=[])

        self.nc.all_engine_barrier()
        assert self.sems is not None
        popped = self.nc._tile_sem_poison_stack.pop()
        assert popped is self._sem_poison
        self.nc.clear_and_free_semaphores(list(self.sems.allocated().values()))
        self.nc.all_engine_barrier()

    tile.TileContext._drain_and_barrier = _split_dab
    tile.TileContext._drain_split_installed = True


_install_drain_split()


def build_nc() -> bass.Bass:
    nc = bass.Bass()

    # Per-core shards (host pre-transposes the small operands for layout).
    xT_ext = nc.declare_dram_parameter("xT", [D, NB], F32, isOutput=False)
    WT_ext = nc.declare_dram_parameter("WT", [D, E], F32, isOutput=False)
    b_ext = nc.declare_dram_parameter("bias", [128, NC_D], F32, isOutput=False)
    enc_ext = nc.declare_dram_parameter("enc", [T, NB, E], F32, isOutput=False)
    out_ext = nc.declare_dram_parameter("out", [NB, 2 * E], F32, isOutput=True)

    with tile.TileContext(nc) as tc:
        with (
            tc.tile_pool(name="sb", bufs=1) as sb,
            tc.tile_pool(name="dram", bufs=1, space="DRAM") as dram_pool,
            tc.tile_pool(name="ps", bufs=1, space="PSUM") as ps,
        ):
            ident = sb.tile([128, 128], F32)
            make_identity(nc, ident[:])

            obs_ps1 = ps.tile([1, 16], F32, tag="obs1")
            obs_ps2 = ps.tile([1, 16], F32, tag="obs2")

            def pe_observe(ap, obs):
                return nc.tensor.matmul(obs[:], lhsT=ap[:, 0:1], rhs=ap[:, 0:16],
                                        start=True, stop=True)

            pe_observe(ident, obs_ps1)  # PE observes the identity producer

            # ---- setup: xp = x @ W.T + b ----------------------------------
            xT_sb = sb.tile([128, NC_D, NB], F32)
            nc.sync.dma_start(
                out=xT_sb[:], in_=xT_ext[:, :].rearrange("(c p) b -> p c b", p=128)
            )
            WT_sb = sb.tile([128, NC_D, E], F32)
            nc.sync.dma_start(
                out=WT_sb[:], in_=WT_ext[:, :].rearrange("(c p) e -> p c e", p=128)
            )
            b_sb = sb.tile([128, NC_D], F32)
            nc.sync.dma_start(out=b_sb[:], in_=b_ext[:, :])

            obs_xt = pe_observe(xT_sb[:, 0, :], obs_ps2)  # PE observes xT DMA
            junk_b = sb.tile([128, 1], F32)
            nc.vector.tensor_copy(junk_b[:], b_sb[:, 0:1])  # DVE observes b DMA

            # xp^T chunks: [128 (e-local), ce, b]
            xpT_sb = sb.tile([128, NC_D, NB], F32)
            ps_xpT = ps.tile([128, NB], F32, tag="ps_xpT")
            for ce in range(NC_D):
                for cd in range(NC_D):
                    mm = nc.tensor.matmul(
                        ps_xpT[:],
                        lhsT=WT_sb[:, cd, ce * 128 : (ce + 1) * 128],
                        rhs=xT_sb[:, cd, :],
                        start=(cd == 0),
                        stop=(cd == NC_D - 1),
                    )
                    if ce == 0 and cd == 0:
                        add_dep_helper(mm.ins, obs_xt.ins, sync=False)
                nc.vector.tensor_scalar_add(
                    xpT_sb[:, ce, :], ps_xpT[:], b_sb[:, ce : ce + 1]
                )

            # out[:, 0:E] = xp (natural layout) via TensorE transposes
            out_tile = sb.tile([NB, 2 * E], F32)
            ps_xp = ps.tile([NB, 128], F32, tag="ps_xp")
            for ce in range(NC_D):
                nc.tensor.transpose(ps_xp[:], xpT_sb[:, ce, :], ident[:])
                # DVE (not ACT) so out_tile has a single producer engine
                nc.vector.tensor_copy(
                    out_tile[:, ce * 128 : (ce + 1) * 128], ps_xp[:]
                )

            # Broadcast xp to all 128 partitions via DRAM bounce with a
            # 0-stride partition dim on the read side.
            xp_dram = dram_pool.tile([NB, E], F32)
            nc.sync.dma_start(out=xp_dram[:], in_=out_tile[:, 0:E])
            xpb = sb.tile([128, NB, E], F32)
            nc.sync.dma_start(out=xpb[:], in_=xp_dram[:].partition_broadcast(128))

            # ---- persistent buffers for the t-tile loop -------------------
            # Write-once column layouts (one column/slice per t-tile) avoid
            # same-engine WAW hazards entirely; enc/prod alternate manually.
            enc_t = [
                [sb.tile([PT, NB // 2, E], F32, name=f"enc{i}h{h}") for h in range(2)]
                for i in range(3)
            ]
            prod = [sb.tile([PT, GRP * E], F32, name=f"prod{i}") for i in range(2)]
            S_all = sb.tile([PT, NT, NB], F32)
            pT_all = sb.tile([NB, NT, PT], F32)
            pm_t = [sb.tile([PT, NB, NB], F32, name=f"pm{i}") for i in range(2)]
            nc.vector.memset(pm_t[0][:], 0.0)  # off-diagonals stay 0 forever
            nc.vector.memset(pm_t[1][:], 0.0)
            jpm = sb.tile([1, NT], F32)
            junk_es = sb.tile([PT, 4 * NT], F32)  # 2 cols/tile used
            junk_ss = sb.tile([NB, NT], F32)
            dummy_all = sb.tile([PT, NT, NB], F32)  # write-once reduce dummies
            jsd = sb.tile([PT, NT * (NB // GRP)], F32)
            jra = sb.tile([1, NT * (NB // GRP)], F32)
            jw = sb.tile([1, 1], F32)
            jns = sb.tile([NB, NT], F32)

            sT_ps = ps.tile([NB, PT], F32, tag="sT")
            p_ps = ps.tile([PT, NB], F32, tag="p")
            ctx_pair = [ps.tile([NB, E], F32, name=f"ctx{i}") for i in range(2)]

            NEGM = sb.tile([NB, NT], F32)   # -m_k per (b, k)
            L_all = sb.tile([NB, NT], F32)  # l_k per (b, k)
            c_store = sb.tile([NB, NT, E], F32)

            # ---- software-pipelined t-tile loop --------------------------
            # head(k): enc DMA + multiplies + reduces for tile k
            # tail(k-1): transposes/softmax/context matmuls for tile k-1,
            # emitted after head(k) so every engine stream has filler work
            # and the cross-engine tail chain hides under head(k+1).
            hist = {}
            tile_handles = {}
            cstore_q = []  # tiles whose ctx_ps copy is still pending

            def emit_tail(k, pace=()):
                eth = enc_t[k % 3]
                ctx_ps = ctx_pair[k % 2]
                st_inst = nc.tensor.transpose(sT_ps[:], S_all[:, k, :], ident[0:PT, 0:PT])
                if cstore_q:
                    # the pending c_store copy must land before this tile's
                    # ctx matmuls reuse ctx_ps; syncing the S transpose on it
                    # both orders the streams and keeps jmm at one wait
                    add_dep_helper(st_inst.ins, cstore_q[-1].ins, sync=True)
                if k >= 1:
                    # DVE observes exp(k-1) (the last sT_ps reader) so the
                    # NEGM reduce carries only the PE wait
                    nc.vector.tensor_copy(jns[:, k : k + 1], pT_all[:, k - 1, 0:1])
                nc.vector.tensor_reduce(
                    out=NEGM[:, k : k + 1], in_=sT_ps[:], axis=AX.X, op=ALU.max,
                    negate=True,
                )
                nc.scalar.activation(junk_ss[:, k : k + 1], sT_ps[:, 0:1], AF.Copy)
                nc.scalar.activation(
                    pT_all[:, k, :], sT_ps[:], AF.Exp,
                    bias=NEGM[:, k : k + 1], scale=1.0,
                    accum_out=L_all[:, k : k + 1],
                )
                nc.tensor.transpose(p_ps[:], pT_all[:, k, :], ident[0:NB, 0:NB])
                pm = pm_t[k % 2]
                if k >= 2:
                    # DVE observes its own k-2 diagonal write so the diag
                    # copy below carries only the PE wait
                    nc.vector.tensor_copy(jpm[:, k : k + 1], pm[0:1, 0, 0:1])
                nc.vector.tensor_copy(
                    pm[:, :, :].rearrange("p a b -> p (a b)")[:, :: NB + 1],
                    p_ps[:],
                )
                jmm = nc.tensor.matmul(
                    ctx_ps[0:1, 0:16],
                    lhsT=eth[0][:, 0, 0:1], rhs=eth[0][:, 0, 0:16],
                    start=True, stop=True,
                )
                add_dep_helper(jmm.ins, st_inst.ins, sync=False)
                # PE observes the second half's DMA before its first use
                jmm2 = nc.tensor.matmul(
                    ctx_ps[0:1, 0:16],
                    lhsT=eth[1][:, 0, 0:1], rhs=eth[1][:, 0, 0:16],
                    start=True, stop=True,
                )
                add_dep_helper(jmm2.ins, jmm.ins, sync=False)
                mm_b7 = last_mm = None
                for b in range(NB):
                    last_mm = nc.tensor.matmul(
                        ctx_ps[:],
                        lhsT=pm[:, b, :],
                        rhs=eth[b // 8][:, b % 8, :],
                        start=(b == 0),
                        stop=(b == NB - 1),
                    )
                    if b == 7:
                        mm_b7 = last_mm
                # HAM keep-warm: tiny ident matmuls paced by the NEXT head's
                # group reduces keep PE activity inside every 3.4us window
                # during its idle span, so context matmuls stay at 2.4 GHz.
                # Their deps resolve before the next tail's own data dep, so
                # they never delay real work.
                for dep in pace:
                    jwm = nc.tensor.matmul(
                        obs_ps1[:], lhsT=ident[:, 0:1], rhs=ident[:, 0:16],
                        start=True, stop=True,
                    )
                    add_dep_helper(jwm.ins, dep.ins, sync=True)
                return mm_b7, last_mm

            def emit_cstore(k):
                inst = nc.scalar.activation(
                    c_store[:, k, :], ctx_pair[k % 2][:], AF.Copy
                )
                cstore_q.append(inst)
                return inst

            for k in range(NT):
                eth = enc_t[k % 3]
                # Pre-absorb each half-buffer's reuse hazards on the Pool
                # proc with explicitly-synced nops, so the SWDGE DMAs need
                # no more than the allowed number of waits.
                dmas = []
                for h in range(2):
                    if k >= 3:
                        for dep in hist[(k % 3, h)]:
                            nop = nc.gpsimd.engine_nop()
                            add_dep_helper(nop.ins, dep.ins, sync=True)
                    d = nc.gpsimd.dma_start(
                        out=eth[h][:],
                        in_=enc_ext[k * PT : (k + 1) * PT, 8 * h : 8 * (h + 1), :],
                    )
                    dmas.append(d)
                    # DVE observes each half's DMA
                    nc.vector.tensor_copy(
                        junk_es[:, 2 * k + h : 2 * k + h + 1], eth[h][:, 0, 0:1]
                    )

                last_tt = None
                gr_reds = []
                ngr = NB // GRP
                for g in range(ngr):
                    pr = prod[g % 2]
                    gcol = k * ngr + g
                    # group 0's first two reduces run on DVE (k>=1) to
                    # balance the engines; the group's last reduce is always
                    # ScalarE so the jsd observer stays single-engine.
                    dve_grp = g == 0 and k >= 1
                    # DVE observes the g-2 reduces (via their S column) so
                    # the multiply's WAR on prod elides; its one remaining
                    # wait is the same-engine WAW, which walrus allows.
                    if k > 0 or g >= 2:
                        pk, pg = (k, g - 2) if g >= 2 else (k - 1, g + ngr - 2)
                        pb = pg * GRP + GRP - 1
                        nc.vector.tensor_copy(
                            jsd[:, gcol : gcol + 1], S_all[:, pk, pb : pb + 1]
                        )
                    # prod = enc * xp for GRP batches (one DVE op)
                    h = (g * GRP) // 8
                    bl = g * GRP - 8 * h
                    last_tt = nc.vector.tensor_tensor(
                        out=pr[:].rearrange("p (g e) -> p g e", g=GRP),
                        in0=eth[h][:, bl : bl + GRP, :],
                        in1=xpb[0:PT, g * GRP : (g + 1) * GRP, :],
                        op=ALU.mult,
                    )
                    if h == 0 and g == (8 // GRP) - 1:
                        half_tt0 = last_tt
                    # ACT observes the fresh multiply so its reduces are
                    # wait-free
                    nc.scalar.activation(
                        jra[0:1, gcol : gcol + 1], pr[0:1, 0:1], AF.Copy
                    )
                    for j in range(GRP):
                        b = g * GRP + j
                        if dve_grp and j < 2:
                            last_red = nc.vector.tensor_reduce(
                                out=S_all[:, k, b : b + 1],
                                in_=pr[:, j * E : (j + 1) * E],
                                axis=AX.X, op=ALU.add,
                            )
                        else:
                            # rowsum via ScalarE activation accumulate; the
                            # dummy output lands on a write-once broadcast col
                            last_red = nc.scalar.activation(
                                dummy_all[:, k, b : b + 1].broadcast_to((PT, E)),
                                pr[:, j * E : (j + 1) * E],
                                AF.Copy,
                                accum_out=S_all[:, k, b : b + 1],
                            )
                    gr_reds.append(last_red)
                    if g == 0 and k >= 2:
                        emit_cstore(k - 2)

                tile_handles[k] = (dmas, half_tt0, last_tt)
                if k >= 1:
                    mm_b7, mm_b15 = emit_tail(k - 1)
                    d2, ht0, t2 = tile_handles[k - 1]
                    hist[((k - 1) % 3, 0)] = (mm_b7, ht0, d2[0])
                    hist[((k - 1) % 3, 1)] = (mm_b15, t2, d2[1])

            emit_cstore(NT - 2)
            emit_tail(NT - 1)
            emit_cstore(NT - 1)
            prev_cstore = cstore_q[-1]

            # ---- final combine across tiles -------------------------------
            negM = sb.tile([NB, 1], F32)
            nc.vector.tensor_reduce(out=negM[:], in_=NEGM[:], axis=AX.X, op=ALU.min)
            alpha = sb.tile([NB, NT], F32)
            # alpha = exp(-NEGM * 1 + (-M)) = exp(m_k - M); ordered after the
            # last c_store copy so the combine loop's ACT waits all elide
            ainst = nc.scalar.activation(
                alpha[:], NEGM[:], AF.Exp, bias=negM[:], scale=-1.0
            )
            add_dep_helper(ainst.ins, prev_cstore.ins, sync=False)
            prodw = sb.tile([NB, NT], F32)
            nc.vector.tensor_tensor(out=prodw[:], in0=alpha[:], in1=L_all[:],
                                    op=ALU.mult)
            Lsum = sb.tile([NB, 1], F32)
            nc.vector.tensor_reduce(out=Lsum[:], in_=prodw[:], axis=AX.X, op=ALU.add)
            rL = sb.tile([NB, 1], F32)
            nc.vector.reciprocal(rL[:], Lsum[:])
            w = sb.tile([NB, NT], F32)
            nc.vector.tensor_scalar_mul(w[:], alpha[:], rL[:])

            acc = out_tile[:, E : 2 * E]
            nc.vector.tensor_copy(jw[:], w[0:1, 0:1])  # absorb w's self-wait
            nc.vector.tensor_scalar_mul(acc, c_store[:, 0, :], w[:, 0:1])
            for k in range(1, NT):
                nc.vector.scalar_tensor_tensor(
                    out=acc, in0=c_store[:, k, :], scalar=w[:, k : k + 1], in1=acc,
                    op0=ALU.mult, op1=ALU.add,
                )

            nc.sync.dma_start(out=out_ext[:, :], in_=out_tile[:])

    return nc


_NC_CACHE: bass.Bass | None = None


def _get_nc() -> bass.Bass:
    global _NC_CACHE
    if _NC_CACHE is None:
        _NC_CACHE = build_nc()
    return _NC_CACHE


def make_in_maps(inputs: dict) -> list[dict]:
    x = np.ascontiguousarray(np.asarray(inputs["x"], dtype=np.float32))
    enc = np.asarray(inputs["encoder_states"], dtype=np.float32)
    W = np.asarray(inputs["W"], dtype=np.float32)
    bias = np.asarray(inputs["b"], dtype=np.float32)

    WT = np.ascontiguousarray(W.T)
    b128 = np.ascontiguousarray(bias.reshape(NC_D, 128).T)
    in_maps = []
    for i in range(NCORES):
        sl = slice(i * NB, (i + 1) * NB)
        in_maps.append(
            {
                "xT": np.ascontiguousarray(x[sl].T),
                "WT": WT,
                "bias": b128,
                "enc": np.ascontiguousarray(enc[:, sl, :]),
            }
        )
    return in_maps


def run(inputs: dict, trace: bool = False, tmpdir: str | None = None):
    """Returns (full_output [B, 2E] f32, exec_time_ns or None)."""
    nc = _get_nc()
    in_maps = make_in_maps(inputs)
    res = run_bass_kernel_spmd(
        nc, in_maps, core_ids=list(range(NCORES)), trace=trace, tmpdir=tmpdir
    )
    out = np.concatenate([res.results[i]["out"] for i in range(NCORES)], axis=0)
    return out.astype(np.float32), res.exec_time_ns


def kernel(**inputs) -> np.ndarray:
    out, _ = run(inputs, trace=False)
    return out



# revision 3
# speedup vs baseline: 1.3285x; 1.3285x over previous
"""Trainium2 Bass kernel for the attention module:

    xp      = x @ W.T + b                      # [B, E]
    scores  = einsum('be,tbe->bt', xp, enc)    # [B, T]
    attn    = softmax(scores, axis=1)
    context = einsum('bt,tbe->be', attn, enc)  # [B, E]
    out     = concat([xp, context], axis=1)    # [B, 2E]

Shapes: T=2048, B=128, D_dec=512, E=512 (fp32).

Strategy (data-parallel over batch, 8 NeuronCores, no collectives):
  - Each core owns NB=16 batches: its encoder_states shard is
    [T, 16, E] = 64 MiB fp32, streamed from HBM exactly once in NT=16
    t-tiles of [128, 16, 512], CAST TO FP16 during the SWDGE DMA
    (HBM read stays fp32 -> memory roofline ~188us; SBUF side halves).
  - fp16 on-chip enc buys: DVE 2x_1p mode for the big multiply, and
    1 cycle/row PE matmuls for the context accumulation (fp32 would be
    4 cycles/row = 2 passes).  Score accumulation stays fp32 (DVE/ACT
    reduce accumulators), softmax numerics preserved.
  - Per tile k (flash-style, deferred softmax combine):
      prod     = enc * xp            (ONE fp16 DVE tensor_tensor)
      S[t,b]   = sum_e prod          (batches 0-8: one batched DVE
                 tensor_reduce; 9-15: ScalarE activation accum)
      sT       = S^T                 (TensorE transpose, fp32)
      -m_k     = -rowmax(sT)         (VectorE reduce)
      pT, l_k  = exp(sT - m_k) + rowsum (ScalarE activation, pT fp16)
      p        = pT^T                (TensorE transpose, fp16)
      c_k      = per-batch sum_t p * enc via 16 masked fp16 matmuls
                 accumulating one [16, 512] fp32 PSUM tile (TensorE)
  - Final: exact softmax combine over the 16 tiles' (m_k, l_k, c_k),
    all fp32.

This toolchain's walrus accepts AT MOST ONE semaphore wait per TPB
compute instruction, and Tile pool slot reuse emits extra release
waits.  Hence: hot buffers are allocated once and alternated manually,
and cheap "observer" ops make each engine see a new producer before
the real consumer runs, keeping every instruction at <= 1 wait.
"""

import os
import sys

import numpy as np

if "/opt/trn_rl_repo" not in sys.path and not any(
    os.path.isdir(os.path.join(p, "concourse")) for p in sys.path if p
):
    sys.path.insert(0, "/opt/trn_rl_repo")

import concourse.bass as bass
import concourse.mybir as mybir
import concourse.tile as tile
from concourse.bass_utils import run_bass_kernel_spmd
from concourse.masks import make_identity
from concourse.tile_rust import add_dep_helper

T, B, D, E = 2048, 128, 512, 512
NCORES = 8
NB = B // NCORES  # 16 local batches per core
PT = 128          # t-tile partition size
NT = T // PT      # 16 t-tiles
NC_D = D // 128   # 4 chunks of the contraction dim for the xp matmul
NBUF = 4          # rotating fp16 enc tile buffers
NDVE = 9          # batches reduced by the batched DVE tensor_reduce

F32 = mybir.dt.float32
F16 = mybir.dt.float16
AF = mybir.ActivationFunctionType
ALU = mybir.AluOpType
AX = mybir.AxisListType


def _install_drain_split():
    """This walrus rejects instructions carrying more than one semaphore
    wait.  Tile's kernel-tail drain waits on every proc's final tick in a
    single instruction; split it into one drain per wait."""
    from concourse.vector_clock import ScopedClock

    if getattr(tile.TileContext, "_drain_split_installed", False):
        return

    def _split_dab(self, tick_clock, wait_clock):
        drain_inst = self.nc.sync.drain()
        wait_clock.add_sem_waits(
            drain_inst.ins, ScopedClock({None: tick_clock.global_clock})
        )
        si = drain_inst.ins.sync_info
        if si is not None and len(si.on_wait) > 1:
            waits = list(si.on_wait)
            upds = list(si.on_update)
            drain_inst.ins.sync_info = mybir.SyncInfo(
                on_wait=[waits[0]], on_update=upds
            )
            for w in waits[1:]:
                d2 = self.nc.sync.drain()
                d2.ins.sync_info = mybir.SyncInfo(on_wait=[w], on_update=[])

        self.nc.all_engine_barrier()
        assert self.sems is not None
        popped = self.nc._tile_sem_poison_stack.pop()
        assert popped is self._sem_poison
        self.nc.clear_and_free_semaphores(list(self.sems.allocated().values()))
        self.nc.all_engine_barrier()

    tile.TileContext._drain_and_barrier = _split_dab
    tile.TileContext._drain_split_installed = True


_install_drain_split()


def build_nc() -> bass.Bass:
    nc = bass.Bass()

    # Per-core shards (host pre-transposes the small operands for layout).
    xT_ext = nc.declare_dram_parameter("xT", [D, NB], F32, isOutput=False)
    WT_ext = nc.declare_dram_parameter("WT", [D, E], F32, isOutput=False)
    b_ext = nc.declare_dram_parameter("bias", [128, NC_D], F32, isOutput=False)
    enc_ext = nc.declare_dram_parameter("enc", [T, NB, E], F32, isOutput=False)
    out_ext = nc.declare_dram_parameter("out", [NB, 2 * E], F32, isOutput=True)

    with tile.TileContext(nc) as tc:
        with (
            tc.tile_pool(name="sb", bufs=1) as sb,
            tc.tile_pool(name="dram", bufs=1, space="DRAM") as dram_pool,
            tc.tile_pool(name="ps", bufs=1, space="PSUM") as ps,
        ):
            ident = sb.tile([128, 128], F32)
            make_identity(nc, ident[:])
            identH = sb.tile([128, 128], F16)
            make_identity(nc, identH[:])

            obs_ps1 = ps.tile([1, 16], F32, tag="obs1")
            obs_ps2 = ps.tile([1, 16], F32, tag="obs2")

            def pe_observe(ap, obs):
                return nc.tensor.matmul(obs[:], lhsT=ap[:, 0:1], rhs=ap[:, 0:16],
                                        start=True, stop=True)

            pe_observe(ident, obs_ps1)   # PE observes the fp32 identity
            pe_observe(identH, obs_ps1)  # PE observes the fp16 identity

            # ---- setup: xp = x @ W.T + b ----------------------------------
            xT_sb = sb.tile([128, NC_D, NB], F32)
            nc.sync.dma_start(
                out=xT_sb[:], in_=xT_ext[:, :].rearrange("(c p) b -> p c b", p=128)
            )
            WT_sb = sb.tile([128, NC_D, E], F32)
            nc.sync.dma_start(
                out=WT_sb[:], in_=WT_ext[:, :].rearrange("(c p) e -> p c e", p=128)
            )
            b_sb = sb.tile([128, NC_D], F32)
            nc.sync.dma_start(out=b_sb[:], in_=b_ext[:, :])

            obs_xt = pe_observe(xT_sb[:, 0, :], obs_ps2)  # PE observes xT DMA
            junk_b = sb.tile([128, 1], F32)
            nc.vector.tensor_copy(junk_b[:], b_sb[:, 0:1])  # DVE observes b DMA

            # xp^T chunks: [128 (e-local), ce, b]
            xpT_sb = sb.tile([128, NC_D, NB], F32)
            ps_xpT = ps.tile([128, NB], F32, tag="ps_xpT")
            for ce in range(NC_D):
                for cd in range(NC_D):
                    mm = nc.tensor.matmul(
                        ps_xpT[:],
                        lhsT=WT_sb[:, cd, ce * 128 : (ce + 1) * 128],
                        rhs=xT_sb[:, cd, :],
                        start=(cd == 0),
                        stop=(cd == NC_D - 1),
                    )
                    if ce == 0 and cd == 0:
                        add_dep_helper(mm.ins, obs_xt.ins, sync=False)
                nc.vector.tensor_scalar_add(
                    xpT_sb[:, ce, :], ps_xpT[:], b_sb[:, ce : ce + 1]
                )

            # out[:, 0:E] = xp (natural layout) via TensorE transposes
            out_tile = sb.tile([NB, 2 * E], F32)
            ps_xp = ps.tile([NB, 128], F32, tag="ps_xp")
            for ce in range(NC_D):
                nc.tensor.transpose(ps_xp[:], xpT_sb[:, ce, :], ident[:])
                # DVE (not ACT) so out_tile has a single producer engine
                nc.vector.tensor_copy(
                    out_tile[:, ce * 128 : (ce + 1) * 128], ps_xp[:]
                )

            # Broadcast xp (cast fp16) to all 128 partitions via DRAM bounce
            # with a 0-stride partition dim on the read side.  The cast
            # happens on the SWDGE write leg; the big broadcast read leg is
            # HWDGE fp16->fp16.
            xp_dram = dram_pool.tile([NB, E], F16)
            nc.gpsimd.dma_start(out=xp_dram[:], in_=out_tile[:, 0:E])
            xpb = sb.tile([128, NB, E], F16)
            nc.sync.dma_start(out=xpb[:], in_=xp_dram[:].partition_broadcast(128))

            # ---- persistent buffers for the t-tile loop -------------------
            # Write-once column layouts (one column/slice per t-tile) avoid
            # same-engine WAW hazards entirely; enc/prod alternate manually.
            enc_t = [sb.tile([PT, NB, E], F16, name=f"enc{i}") for i in range(NBUF)]
            prod = [sb.tile([PT, NB, E], F16, name=f"prod{i}") for i in range(2)]
            S_all = sb.tile([PT, NT, NB], F32)
            pT_all = sb.tile([NB, NT, PT], F16)
            pm_t = [sb.tile([PT, NB, NB], F16, name=f"pm{i}") for i in range(2)]
            nc.vector.memset(pm_t[0][:], 0.0)  # off-diagonals stay 0 forever
            nc.vector.memset(pm_t[1][:], 0.0)
            jpm = sb.tile([1, NT], F32)
            junk_es = sb.tile([PT, NT], F32)
            junk_ss = sb.tile([NB, NT], F32)
            dummy_all = sb.tile([PT, NT, NB], F32)  # write-once reduce dummies
            jw = sb.tile([1, 1], F32)
            jns = sb.tile([NB, NT], F32)

            sT_ps = ps.tile([NB, PT], F32, tag="sT")
            p_ps = ps.tile([PT, NB], F16, tag="p")
            ctx_pair = [ps.tile([NB, E], F32, name=f"ctx{i}") for i in range(2)]

            NEGM = sb.tile([NB, NT], F32)   # -m_k per (b, k)
            L_all = sb.tile([NB, NT], F32)  # l_k per (b, k)
            c_store = sb.tile([NB, NT, E], F32)

            # ---- software-pipelined t-tile loop --------------------------
            # head(k): enc DMA + multiply + reduces for tile k
            # tail(k-1): transposes/softmax/context matmuls for tile k-1,
            # emitted after head(k) so every engine stream has filler work
            # and the cross-engine tail chain hides under head(k+1).
            hist = {}
            handles = {}
            cstore_q = []  # tiles whose ctx_ps copy is still pending

            def emit_tail(k):
                eth = enc_t[k % NBUF]
                ctx_ps = ctx_pair[k % 2]
                st_inst = nc.tensor.transpose(sT_ps[:], S_all[:, k, :], ident[0:PT, 0:PT])
                if cstore_q:
                    # the pending c_store copy must land before this tile's
                    # ctx matmuls reuse ctx_ps; syncing the S transpose on it
                    # both orders the streams and keeps jmm at one wait
                    add_dep_helper(st_inst.ins, cstore_q[-1].ins, sync=True)
                if k >= 1:
                    # DVE observes exp(k-1) (the last sT_ps reader) so the
                    # NEGM reduce carries only the PE wait
                    nc.vector.tensor_copy(jns[:, k : k + 1], pT_all[:, k - 1, 0:1])
                nc.vector.tensor_reduce(
                    out=NEGM[:, k : k + 1], in_=sT_ps[:], axis=AX.X, op=ALU.max,
                    negate=True,
                )
                nc.scalar.activation(junk_ss[:, k : k + 1], sT_ps[:, 0:1], AF.Copy)
                nc.scalar.activation(
                    pT_all[:, k, :], sT_ps[:], AF.Exp,
                    bias=NEGM[:, k : k + 1], scale=1.0,
                    accum_out=L_all[:, k : k + 1],
                )
                nc.tensor.transpose(p_ps[:], pT_all[:, k, :], identH[0:NB, 0:NB])
                pm = pm_t[k % 2]
                if k >= 2:
                    # DVE observes its own k-2 diagonal write so the diag
                    # copy below carries only the PE wait
                    nc.vector.tensor_copy(jpm[:, k : k + 1], pm[0:1, 0, 0:1])
                nc.vector.tensor_copy(
                    pm[:, :, :].rearrange("p a b -> p (a b)")[:, :: NB + 1],
                    p_ps[:],
                )
                # PE observes the enc DMA before the ctx matmuls use it
                jmm = nc.tensor.matmul(
                    ctx_ps[0:1, 0:16],
                    lhsT=eth[:, 0, 0:1], rhs=eth[:, 0, 0:16],
                    start=True, stop=True,
                )
                add_dep_helper(jmm.ins, st_inst.ins, sync=False)
                last_mm = None
                for b in range(NB):
                    last_mm = nc.tensor.matmul(
                        ctx_ps[:],
                        lhsT=pm[:, b, :],
                        rhs=eth[:, b, :],
                        start=(b == 0),
                        stop=(b == NB - 1),
                    )
                return last_mm

            def emit_cstore(k):
                inst = nc.scalar.activation(
                    c_store[:, k, :], ctx_pair[k % 2][:], AF.Copy
                )
                cstore_q.append(inst)
                return inst

            for k in range(NT):
                eth = enc_t[k % NBUF]
                pr = prod[k % 2]
                # Pre-absorb the buffer's reuse hazards on the Pool proc
                # with explicitly-synced nops, so the SWDGE DMA needs no
                # more than the allowed number of waits.
                if k >= NBUF:
                    for dep in hist[k % NBUF]:
                        nop = nc.gpsimd.engine_nop()
                        add_dep_helper(nop.ins, dep.ins, sync=True)
                d = nc.gpsimd.dma_start(
                    out=eth[:], in_=enc_ext[k * PT : (k + 1) * PT, :, :]
                )
                # DVE observes the DMA so the multiply's enc wait elides
                nc.vector.tensor_copy(junk_es[:, k : k + 1], eth[:, 0, 0:1])

                # prod = enc * xp for all 16 batches (one fp16 DVE op).
                # Its one cross-engine wait (WAR vs tile k-2's ACT reduces
                # of prod) elides via the jns observer chain.
                tt = nc.vector.tensor_tensor(
                    out=pr[:], in0=eth[:], in1=xpb[:], op=ALU.mult,
                )

                # Scores: batched DVE reduce for the first NDVE batches
                # (tile 0: all 16, so S_all[:,0,:] has a single producer
                # engine and the first tail's transpose carries one wait),
                # ScalarE activation-accumulate for the rest.
                nd = NB if k == 0 else NDVE
                nc.vector.tensor_reduce(
                    out=S_all[:, k, 0:nd], in_=pr[:, 0:nd, :], axis=AX.X,
                    op=ALU.add,
                )
                for i, b in enumerate(range(nd, NB)):
                    nc.scalar.activation(
                        dummy_all[:, k, b : b + 1].broadcast_to((PT, E)),
                        pr[:, b, :],
                        AF.Copy,
                        accum_out=S_all[:, k, b : b + 1],
                    )
                    if i == 0 and k >= 2:
                        emit_cstore(k - 2)

                handles[k] = (d, tt)
                if k >= 1:
                    mm_last = emit_tail(k - 1)
                    d2, tt2 = handles[k - 1]
                    hist[(k - 1) % NBUF] = (mm_last, tt2, d2)

            emit_cstore(NT - 2)
            emit_tail(NT - 1)
            emit_cstore(NT - 1)
            prev_cstore = cstore_q[-1]

            # ---- final combine across tiles -------------------------------
            negM = sb.tile([NB, 1], F32)
            nc.vector.tensor_reduce(out=negM[:], in_=NEGM[:], axis=AX.X, op=ALU.min)
            alpha = sb.tile([NB, NT], F32)
            # alpha = exp(-NEGM * 1 + (-M)) = exp(m_k - M); ordered after the
            # last c_store copy so the combine loop's ACT waits all elide
            ainst = nc.scalar.activation(
                alpha[:], NEGM[:], AF.Exp, bias=negM[:], scale=-1.0
            )
            add_dep_helper(ainst.ins, prev_cstore.ins, sync=False)
            prodw = sb.tile([NB, NT], F32)
            nc.vector.tensor_tensor(out=prodw[:], in0=alpha[:], in1=L_all[:],
                                    op=ALU.mult)
            Lsum = sb.tile([NB, 1], F32)
            nc.vector.tensor_reduce(out=Lsum[:], in_=prodw[:], axis=AX.X, op=ALU.add)
            rL = sb.tile([NB, 1], F32)
            nc.vector.reciprocal(rL[:], Lsum[:])
            w = sb.tile([NB, NT], F32)
            nc.vector.tensor_scalar_mul(w[:], alpha[:], rL[:])

            acc = out_tile[:, E : 2 * E]
            nc.vector.tensor_copy(jw[:], w[0:1, 0:1])  # absorb w's self-wait
            nc.vector.tensor_scalar_mul(acc, c_store[:, 0, :], w[:, 0:1])
            for k in range(1, NT):
                nc.vector.scalar_tensor_tensor(
                    out=acc, in0=c_store[:, k, :], scalar=w[:, k : k + 1], in1=acc,
                    op0=ALU.mult, op1=ALU.add,
                )

            nc.sync.dma_start(out=out_ext[:, :], in_=out_tile[:])

    return nc


_NC_CACHE: bass.Bass | None = None


def _get_nc() -> bass.Bass:
    global _NC_CACHE
    if _NC_CACHE is None:
        _NC_CACHE = build_nc()
    return _NC_CACHE


def make_in_maps(inputs: dict) -> list[dict]:
    x = np.ascontiguousarray(np.asarray(inputs["x"], dtype=np.float32))
    enc = np.asarray(inputs["encoder_states"], dtype=np.float32)
    W = np.asarray(inputs["W"], dtype=np.float32)
    bias = np.asarray(inputs["b"], dtype=np.float32)

    WT = np.ascontiguousarray(W.T)
    b128 = np.ascontiguousarray(bias.reshape(NC_D, 128).T)
    in_maps = []
    for i in range(NCORES):
        sl = slice(i * NB, (i + 1) * NB)
        in_maps.append(
            {
                "xT": np.ascontiguousarray(x[sl].T),
                "WT": WT,
                "bias": b128,
                "enc": np.ascontiguousarray(enc[:, sl, :]),
            }
        )
    return in_maps


def run(inputs: dict, trace: bool = False, tmpdir: str | None = None):
    """Returns (full_output [B, 2E] f32, exec_time_ns or None)."""
    nc = _get_nc()
    in_maps = make_in_maps(inputs)
    res = run_bass_kernel_spmd(
        nc, in_maps, core_ids=list(range(NCORES)), trace=trace, tmpdir=tmpdir
    )
    out = np.concatenate([res.results[i]["out"] for i in range(NCORES)], axis=0)
    return out.astype(np.float32), res.exec_time_ns


def kernel(**inputs) -> np.ndarray:
    out, _ = run(inputs, trace=False)
    return out
